# revision 1
# baseline (speedup 1.0000x reference)
"""Multiview dynamic window attention (Swin-V2 style CPB), data-parallel
over the window*batch axis across 8 NeuronCores.

Self-contained: hardcodes shapes from the problem spec.
  x1, x2: [512, 64, 256] fp32; all params small + replicated.
Sharding: B_=512 split 8 ways (64 windows/core); bias-table computation
is replicated per device. Output: stacked (o1, o2) -> [2, 512, 64, 256].
"""
import numpy as np
import jax
import jax.numpy as jnp

WH, WW = 8, 8
NH = 8
DIM = 256
B_ = 512
N = WH * WW
HD = DIM // NH
NCORES = 8


def _rel_constants():
    ch = np.arange(-(WH - 1), WH, dtype=np.float32) / (WH - 1)
    cw = np.arange(-(WW - 1), WW, dtype=np.float32) / (WW - 1)
    table = np.stack(np.meshgrid(ch, cw, indexing='ij'), axis=-1)[None] * 8.0
    table = np.sign(table) * np.log2(np.abs(table) + 1.0) / np.log2(8.0)
    coords = np.stack(np.meshgrid(np.arange(WH), np.arange(WW), indexing='ij'))
    cf = coords.reshape(2, -1)
    rel = (cf[:, :, None] - cf[:, None, :]).transpose(1, 2, 0).astype(np.int64)
    rel[..., 0] += WH - 1
    rel[..., 1] += WW - 1
    rel[..., 0] *= 2 * WW - 1
    idx = rel.sum(-1)
    return table.astype(np.float32), idx.astype(np.int32)


_TABLE, _IDX = _rel_constants()
_ONEHOT = np.zeros(((2 * WH - 1) * (2 * WW - 1), N * N), dtype=np.float32)
_ONEHOT[_IDX.reshape(-1), np.arange(N * N)] = 1.0  # [225, N*N] gather matrix


def _qkv(x, w, qb, vb):
    b = jnp.concatenate([qb, jnp.zeros_like(vb), vb])
    y = (x @ w.T + b).reshape(x.shape[0], N, 3, NH, HD).transpose(2, 0, 3, 1, 4)
    return y[0], y[1], y[2]


def _l2norm(t):
    n = jnp.linalg.norm(t, axis=-1, keepdims=True)
    return t / jnp.maximum(n, 1e-12)


def _view(qn, k_same_n, k_diff_n, v, logit_scale, pc_w, pc_b, cw1, cb1, cw2,
          proj_w, proj_b, table, onehot):
    same = jnp.einsum('bhnd,bhmd->bhnm', qn, k_same_n)
    diff = jnp.einsum('bhnd,bhmd->bhnm', qn, k_diff_n)
    cat = jnp.concatenate([same, diff], axis=1).transpose(0, 2, 3, 1)
    scale = jnp.exp(jnp.minimum(logit_scale, jnp.log(100.0)))
    attn = (cat @ pc_w.T + pc_b).transpose(0, 3, 1, 2) * scale
    # CPB MLP -> bias table -> gather (as constant one-hot matmul: no
    # dynamic gather needed on device)
    tbl = (jax.nn.relu(table @ cw1.T + cb1) @ cw2.T).reshape(-1, NH)  # [225,H]
    bias = (tbl.T @ onehot).reshape(NH, N, N)  # [H,N,N]
    attn = attn + (16.0 * jax.nn.sigmoid(bias))[None]
    attn = jax.nn.softmax(attn, axis=-1)
    out = jnp.einsum('bhnm,bhmd->bhnd', attn, v).transpose(0, 2, 1, 3).reshape(-1, N, DIM)
    return out @ proj_w.T + proj_b


def _shard_fn(x1, x2, params, table, onehot):
    (qkv_w1, q_bias_1, v_bias_1, proj_w1, proj_b1, logit_scale_1,
     cpb1_w1, cpb1_b1, cpb1_w2, pc1_w, pc1_b,
     qkv_w2, q_bias_2, v_bias_2, proj_w2, proj_b2, logit_scale_2,
     cpb2_w1, cpb2_b1, cpb2_w2, pc2_w, pc2_b) = params
    q1, k1, v1 = _qkv(x1, qkv_w1, q_bias_1, v_bias_1)
    q2, k2, v2 = _qkv(x2, qkv_w2, q_bias_2, v_bias_2)
    q1n, k1n = _l2norm(q1), _l2norm(k1)
    q2n, k2n = _l2norm(q2), _l2norm(k2)
    o1 = _view(q1n, k1n, k2n, v1, logit_scale_1, pc1_w, pc1_b,
               cpb1_w1, cpb1_b1, cpb1_w2, proj_w1, proj_b1, table, onehot)
    o2 = _view(q2n, k2n, k1n, v2, logit_scale_2, pc2_w, pc2_b,
               cpb2_w1, cpb2_b1, cpb2_w2, proj_w2, proj_b2, table, onehot)
    return o1, o2


_pmapped = jax.pmap(_shard_fn, in_axes=(0, 0, None, None, None))


def kernel(**inputs):
    x1 = np.asarray(inputs['x1'], dtype=np.float32)
    x2 = np.asarray(inputs['x2'], dtype=np.float32)
    param_names = [
        'qkv_w1', 'q_bias_1', 'v_bias_1', 'proj_w1', 'proj_b1', 'logit_scale_1',
        'cpb1_w1', 'cpb1_b1', 'cpb1_w2', 'pc1_w', 'pc1_b',
        'qkv_w2', 'q_bias_2', 'v_bias_2', 'proj_w2', 'proj_b2', 'logit_scale_2',
        'cpb2_w1', 'cpb2_b1', 'cpb2_w2', 'pc2_w', 'pc2_b',
    ]
    params = tuple(np.asarray(inputs[k], dtype=np.float32) for k in param_names)

    shard = B_ // NCORES
    x1s = x1.reshape(NCORES, shard, N, DIM)
    x2s = x2.reshape(NCORES, shard, N, DIM)

    o1s, o2s = _pmapped(x1s, x2s, params, _TABLE, _ONEHOT)
    o1 = np.asarray(o1s).reshape(B_, N, DIM)
    o2 = np.asarray(o2s).reshape(B_, N, DIM)
    return np.stack([o1, o2], axis=0)


# revision 2
# speedup vs baseline: 1.0809x; 1.0809x over previous
"""Multiview dynamic window attention (Swin-V2 style CPB), data-parallel
over the window*batch axis across 8 NeuronCores.

Self-contained: hardcodes shapes from the problem spec.
  x1, x2: [512, 64, 256] fp32; all params small + replicated.
Sharding: B_=512 split 8 ways (64 windows/core). The CPB-MLP bias table
is parameter-only (no data dependence), so it is computed once on host
and shipped as an 8KB per-view bias — the axon tunnel is ~43MB/s, so
shipping the gather matrix to all 8 cores each call would dominate.
Output: stacked (o1, o2) -> [2, 512, 64, 256].
"""
import numpy as np
import jax
import jax.numpy as jnp

WH, WW = 8, 8
NH = 8
DIM = 256
B_ = 512
N = WH * WW
HD = DIM // NH
NCORES = 8


def _rel_constants():
    ch = np.arange(-(WH - 1), WH, dtype=np.float32) / (WH - 1)
    cw = np.arange(-(WW - 1), WW, dtype=np.float32) / (WW - 1)
    table = np.stack(np.meshgrid(ch, cw, indexing='ij'), axis=-1)[None] * 8.0
    table = np.sign(table) * np.log2(np.abs(table) + 1.0) / np.log2(8.0)
    coords = np.stack(np.meshgrid(np.arange(WH), np.arange(WW), indexing='ij'))
    cf = coords.reshape(2, -1)
    rel = (cf[:, :, None] - cf[:, None, :]).transpose(1, 2, 0).astype(np.int64)
    rel[..., 0] += WH - 1
    rel[..., 1] += WW - 1
    rel[..., 0] *= 2 * WW - 1
    idx = rel.sum(-1)
    return table.astype(np.float32), idx


_TABLE, _IDX = _rel_constants()


def _host_bias(cw1, cb1, cw2):
    """16*sigmoid(CPB-MLP table gathered by IDX) -> [NH, N, N], fp32."""
    h = np.maximum(_TABLE @ cw1.T + cb1, 0.0)
    tbl = (h @ cw2.T).reshape(-1, NH)
    bias = tbl[_IDX.reshape(-1)].reshape(N, N, NH).transpose(2, 0, 1)
    return (16.0 / (1.0 + np.exp(-bias))).astype(np.float32)


def _qkv(x, w, qb, vb):
    b = jnp.concatenate([qb, jnp.zeros_like(vb), vb])
    y = (x @ w.T + b).reshape(x.shape[0], N, 3, NH, HD).transpose(2, 0, 3, 1, 4)
    return y[0], y[1], y[2]


def _l2norm(t):
    n = jnp.linalg.norm(t, axis=-1, keepdims=True)
    return t / jnp.maximum(n, 1e-12)


def _view(qn, k_same_n, k_diff_n, v, pc_ws, pc_bs, bias16, proj_w, proj_b):
    # pc_ws/pc_bs already folded with the clamped logit scale on host;
    # bias16 = 16*sigmoid(CPB table) precomputed on host.
    same = jnp.einsum('bhnd,bhmd->bhnm', qn, k_same_n)
    diff = jnp.einsum('bhnd,bhmd->bhnm', qn, k_diff_n)
    cat = jnp.concatenate([same, diff], axis=1).transpose(0, 2, 3, 1)
    attn = (cat @ pc_ws.T + pc_bs).transpose(0, 3, 1, 2)
    attn = attn + bias16[None]
    attn = jax.nn.softmax(attn, axis=-1)
    out = jnp.einsum('bhnm,bhmd->bhnd', attn, v).transpose(0, 2, 1, 3).reshape(-1, N, DIM)
    return out @ proj_w.T + proj_b


def _shard_fn(x1, x2, params):
    (qkv_w1, q_bias_1, v_bias_1, proj_w1, proj_b1, pc1_ws, pc1_bs, bias16_1,
     qkv_w2, q_bias_2, v_bias_2, proj_w2, proj_b2, pc2_ws, pc2_bs, bias16_2) = params
    q1, k1, v1 = _qkv(x1, qkv_w1, q_bias_1, v_bias_1)
    q2, k2, v2 = _qkv(x2, qkv_w2, q_bias_2, v_bias_2)
    q1n, k1n = _l2norm(q1), _l2norm(k1)
    q2n, k2n = _l2norm(q2), _l2norm(k2)
    o1 = _view(q1n, k1n, k2n, v1, pc1_ws, pc1_bs, bias16_1, proj_w1, proj_b1)
    o2 = _view(q2n, k2n, k1n, v2, pc2_ws, pc2_bs, bias16_2, proj_w2, proj_b2)
    return o1, o2


_pmapped = jax.pmap(_shard_fn, in_axes=(0, 0, None))


def kernel(**inputs):
    f32 = lambda k: np.asarray(inputs[k], dtype=np.float32)
    x1, x2 = f32('x1'), f32('x2')

    # host-side parameter folding (all tiny)
    def fold(ls, pw, pb):
        scale = np.exp(np.minimum(np.asarray(ls, np.float64), np.log(100.0)))
        scale = scale.reshape(NH, 1).astype(np.float32)  # [NH,1]
        return (np.asarray(pw, np.float32) * scale).astype(np.float32), \
               (np.asarray(pb, np.float32) * scale[:, 0]).astype(np.float32)

    pc1_ws, pc1_bs = fold(inputs['logit_scale_1'], inputs['pc1_w'], inputs['pc1_b'])
    pc2_ws, pc2_bs = fold(inputs['logit_scale_2'], inputs['pc2_w'], inputs['pc2_b'])
    bias16_1 = _host_bias(f32('cpb1_w1'), f32('cpb1_b1'), f32('cpb1_w2'))
    bias16_2 = _host_bias(f32('cpb2_w1'), f32('cpb2_b1'), f32('cpb2_w2'))

    params = (f32('qkv_w1'), f32('q_bias_1'), f32('v_bias_1'),
              f32('proj_w1'), f32('proj_b1'), pc1_ws, pc1_bs, bias16_1,
              f32('qkv_w2'), f32('q_bias_2'), f32('v_bias_2'),
              f32('proj_w2'), f32('proj_b2'), pc2_ws, pc2_bs, bias16_2)

    shard = B_ // NCORES
    x1s = x1.reshape(NCORES, shard, N, DIM)
    x2s = x2.reshape(NCORES, shard, N, DIM)

    o1s, o2s = _pmapped(x1s, x2s, params)
    o1 = np.asarray(o1s).reshape(B_, N, DIM)
    o2 = np.asarray(o2s).reshape(B_, N, DIM)
    return np.stack([o1, o2], axis=0)
